# revision 53
# baseline (speedup 1.0000x reference)
"""Trainium2 Bass kernel for ChunkedLocalSelfAttention.

Module: x[B,C,H,W] -> qkv proj -> 8-head local-window attention (17x17
spatial window) -> out proj -> +residual -> 1x1 conv -> relu.
B,C,H,W = 4,256,48,48; N = 2304 tokens per image; head dim 32.

Sharding: 8 cores = 4 batch images x 2 query-row-halves (24 rows each).
Attention output rows only depend on +-8 image rows, so cores need no
communication; the row halo is covered by a 32-row k/v band.

On-core design (scores kept TRANSPOSED: keys on partitions, queries free):
  - 2D key chunking: key chunks are [8 rows x 16 cols] = 128 keys, so the
    score block for a chunk only spans the queries within the +-8 COLUMN
    window of its strip ([8 q-rows x <=32 q-cols] instead of all 48 cols).
    This cuts exp/mask/score/PV element count ~30% vs row-major chunks.
  - k and v are projected from a chunk-major-permuted copy of x (xcm) so
    each chunk's 128 keys are contiguous (matmul stationary APs must have
    a single free dim); q is projected from a row-major copy.
  - k-bias is dropped entirely (constant-per-query terms cancel in
    softmax); q-bias is added during the PSUM->SBUF cast; v-bias is folded
    into the residual on the host (softmax weights sum to 1).
  - score chunks are processed in PAIRS sharing one PSUM bank per head
    (packed 2D crops), halving Activation-engine op count: exp runs on
    [128, 2 heads, E_pair<=448] per op.
  - binary window mask applied multiplicatively AFTER exp via TensorTensor
    (DVE 2x mode: bf16, packed, SBUF); v PSUM->SBUF casts run on the
    Activation engine to balance DVE vs Act load.
  - PV += v_chunk.T @ masked per chunk with [v|ones] packed lhsT (M=64);
    the ones columns replicate each head's softmax denominator.
  - normalize: reciprocal (DVE), 32-partition shift (DMA), multiply (DVE),
    compact to channel order (DMA).
  - out proj, +residual(+out_b+Wo@bv folded on host), 1x1 conv, relu.
  - inputs packed into 3 wide DMAs (transfers serialize in hw); phase-1
    projection chunks woven into qt0/qt1's attention stream.
"""

import sys

for _p in ("/opt/trn_rl_repo",):
    if _p not in sys.path:
        sys.path.insert(0, _p)

import math

import ml_dtypes
import numpy as np

B, C, H, W = 4, 256, 48, 48
N = H * W
HEADS, HD, HALF = 8, 32, 8
NCORES = 8
ROWS_HALF = H // 2          # 24 query rows per core
NQ = ROWS_HALF * W          # 1152 queries per core
BAND_ROWS = 32              # k/v row band per core (24 + 8 halo)
NKCM = BAND_ROWS * W        # 1536 band tokens, chunk-major
QT = 384                    # queries per tile (8 image rows)
SCALE = 1.0 / math.sqrt(HD)

# 2D chunk geometry: chunks are [8 rows x 16 cols]; strips at cols 0/16/32.
CLO = (0, 8, 24)            # query-col crop per strip
CHI = (24, 40, 48)
NCS = (24, 32, 24)
ES = tuple(8 * n for n in NCS)   # 192, 256, 192

bf16 = ml_dtypes.bfloat16

_PROG = None


def _chunks(qt):
    """[(chunk_id, strip)] for query tile qt; chunk_id indexes the 12
    [8x16] chunks of the 32-row band (band*3 + strip)."""
    nb = 2 if qt == 0 else 3
    boff = (0, 0, 1)[qt]
    return [((br + boff) * 3 + s, s) for br in range(nb) for s in range(3)]


def _pairs(qt):
    ch = _chunks(qt)
    return [ch[i : i + 2] for i in range(0, len(ch), 2)]


def _deltas(qt):
    return (0, 8) if qt == 0 else (-8, 0, 8)


def _mask_offsets():
    """Column offsets of each (kind, op) pair-mask in the packed table."""
    offs = {}
    col = 0
    for kind, deltas in ((0, (0, 8)), (1, (-8, 0, 8))):
        ch = [(d, s) for d in deltas for s in range(3)]
        grps = [ch[i : i + 2] for i in range(0, len(ch), 2)]
        for op_i, grp in enumerate(grps):
            offs[(kind, op_i)] = col
            col += sum(ES[s] for _, s in grp)
    return offs, col


MOFF, MTOT = _mask_offsets()


def _build_program():
    import concourse.bass as bass
    import concourse.mybir as mybir
    import concourse.tile as tile
    from concourse import bacc
    from concourse.alu_op_type import AluOpType as OP

    f32 = mybir.dt.float32
    bft = mybir.dt.bfloat16
    AF = mybir.ActivationFunctionType

    nc = bacc.Bacc(
        "TRN2", target_bir_lowering=False, debug=False, num_devices=NCORES
    )

    def din(name, shape, dt=bft):
        return nc.dram_tensor(name, shape, dt, kind="ExternalInput").ap()

    # inputs are packed host-side into 3 wide [128, x] tensors (both C-halves
    # side by side in the free dim) so each load is ONE DMA and the q path
    # lands in the first ~1.3us transfer:
    #   qpk  [128, 3328] = wqk0(512) | wqk1(512) | xq0(1152) | xq1(1152)
    #   kv   [128, 3584] = xcm0(1536) | xcm1(1536) | wv0(256) | wv1(256)
    #   late [128, 3328] = xres0 | xres1 | wo0 | wo1 | wc0 | wc1
    qpk_d = din("qpk", [128, 3328])
    kv_d = din("kv", [128, 3584])
    late_d = din("late", [128, 3328])
    bias_d = din("biases", [128, 4], f32)   # bq(2) | bcrep(2)
    mask_d = din("masks", [128, MTOT])
    out_d = nc.dram_tensor("out", [C, NQ], f32, kind="ExternalOutput").ap()

    # SPMD trick: one program serves both row-halves. The host ships half-1
    # images VERTICALLY FLIPPED (attention is equivariant under a row flip),
    # so every core sees half-0 geometry: query rows [0,24), band [0,32).

    with tile.TileContext(nc) as tc:
        import contextlib

        ctx = contextlib.ExitStack()
        with ctx:
            cpool = ctx.enter_context(tc.tile_pool(name="const", bufs=1))
            qkpool = ctx.enter_context(tc.tile_pool(name="qk", bufs=1))
            vpool = ctx.enter_context(tc.tile_pool(name="v", bufs=1))
            epool = ctx.enter_context(tc.tile_pool(name="exp", bufs=6))
            apool = ctx.enter_context(tc.tile_pool(name="attn", bufs=6))
            rpool = ctx.enter_context(tc.tile_pool(name="recip", bufs=4))
            opool = ctx.enter_context(tc.tile_pool(name="outb", bufs=3))
            psA = ctx.enter_context(
                tc.tile_pool(name="psA", bufs=2, space="PSUM")
            )
            psPP = ctx.enter_context(
                tc.tile_pool(name="psPP", bufs=2, space="PSUM")
            )
            psB = ctx.enter_context(
                tc.tile_pool(name="psB", bufs=2, space="PSUM")
            )

            # ---- constants / inputs to SBUF ----
            cqpk = cpool.tile([128, 3328], bft, tag="cqpk", name="cqpk")
            ckv = cpool.tile([128, 3584], bft, tag="ckv", name="ckv")
            clate = cpool.tile([128, 3328], bft, tag="clate", name="clate")
            wqk = [cqpk[:, 512 * t : 512 * t + 512] for t in range(2)]

            # startup-critical columns packed first: qpk = wqk(1024) |
            # xq0[:384] | xq1[:384] | xq0[384:] | xq1[384:];
            # kv = xcm0[:512] | xcm1[:512] | wv0 | wv1 | xcm0[512:] | xcm1[512:]
            def xqv(cc, n0):
                if n0 == 0:
                    return cqpk[:, 1024 + 384 * cc : 1024 + 384 * cc + QT]
                return cqpk[:, 1792 + 768 * cc + (n0 - 384) :
                            1792 + 768 * cc + (n0 - 384) + QT]

            def xcmv(cc, n0, w):
                if n0 + w <= 512:
                    return ckv[:, 512 * cc + n0 : 512 * cc + n0 + w]
                return ckv[:, 1536 + 1024 * cc + (n0 - 512) :
                           1536 + 1024 * cc + (n0 - 512) + w]

            wv = [ckv[:, 1024 + 256 * t : 1024 + 256 * t + 256] for t in range(2)]
            xres = [clate[:, NQ * t : NQ * t + NQ] for t in range(2)]
            wo = [clate[:, 2304 + 256 * t : 2304 + 256 * t + 256] for t in range(2)]
            wc = [clate[:, 2816 + 256 * t : 2816 + 256 * t + 256] for t in range(2)]
            biases = cpool.tile([128, 4], f32, tag="biases")
            bq = biases[:, 0:2]
            bcr = biases[:, 2:4]
            zrow = cpool.tile([1, 512], bft, tag="zrow")
            nc.gpsimd.memset(zrow[:], 0.0)
            msk = cpool.tile([128, MTOT], bft, tag="msk")
            # preload the Exp table while DMAs are in flight
            actwarm = cpool.tile([1, 8], bft, tag="actwarm")
            nc.scalar.activation(actwarm[:], zrow[0:1, 0:8], AF.Exp)

            def pe_warm(n):
                # dependency-free zero-matmuls keep the PE p-state ramped
                # while it would otherwise idle (cold matmuls run 2-3.7x
                # slower); they slot into idle PE time ahead of real work.
                for _ in range(n):
                    w = psA.tile([128, 1024], f32, tag="sc", name="sc")
                    nc.tensor.matmul(
                        w[:, 0:QT],
                        lhsT=zrow[:, 0:128],
                        rhs=zrow[:, 0:QT],
                        start=True,
                        stop=True,
                        skip_group_check=True,
                    )

            pe_warm(4)
            # loads in first-use order (the transfers serialize): small
            # prefix DMAs carry exactly what qt0 pair-0 op-0 needs, the
            # rest streams in behind them.
            nc.sync.dma_start(cqpk[:, 0:1792], qpk_d[:, 0:1792])
            nc.sync.dma_start(ckv[:, 0:1536], kv_d[:, 0:1536])
            nc.sync.dma_start(biases[:], bias_d[:])
            nc.sync.dma_start(msk[:, 0:448], mask_d[:, 0:448])
            nc.sync.dma_start(ckv[:, 1536:3584], kv_d[:, 1536:3584])
            nc.sync.dma_start(msk[:, 448:1280], mask_d[:, 448:1280])
            nc.sync.dma_start(cqpk[:, 1792:3328], qpk_d[:, 1792:3328])
            nc.sync.dma_start(clate[:], late_d[:])
            nc.sync.dma_start(msk[:, 1280:MTOT], mask_d[:, 1280:MTOT])

            # ---- phase 1: projections ----
            # q tiles [128ch, 1152 row-major]; k tiles [128ch, 1536 chunk-major]
            qkq = [qkpool.tile([128, NQ], bft, tag=f"qkq{i}", name=f"qkq{i}") for i in range(2)]
            qkk = [qkpool.tile([128, NKCM], bft, tag=f"qkk{i}", name=f"qkk{i}") for i in range(2)]
            # v tiles per chunk: head h cols [64h,64h+32)=v_h, [64h+32,64h+64)=1
            vt = [vpool.tile([128, 8 * 64], bft, tag=f"v{i}", name=f"v{i}") for i in range(12)]
            for i in range(12):
                va = vt[i][:].rearrange("p (h two v) -> p h two v", two=2, v=32)
                nc.gpsimd.memset(va[:, :, 1, :], 1.0)

            def q_proj(qc, tiles=(0, 384, 768)):
                for n0 in tiles:
                    ps = psB.tile([128, 512], f32, tag="ps", name="ps")
                    for cc in range(2):
                        nc.tensor.matmul(
                            ps[:, :QT],
                            lhsT=wqk[cc][:, 128 * qc : 128 * qc + 128],
                            rhs=xqv(cc, n0),
                            start=(cc == 0),
                            stop=(cc == 1),
                        )
                    nc.vector.tensor_scalar_add(
                        qkq[qc][:, n0 : n0 + QT], ps[:, :QT], bq[:, qc : qc + 1]
                    )

            def k_proj(kc, tiles=(0, 512, 1024)):
                for n0 in tiles:
                    ps = psB.tile([128, 512], f32, tag="ps", name="ps")
                    for cc in range(2):
                        nc.tensor.matmul(
                            ps[:],
                            lhsT=wqk[cc][:, 256 + 128 * kc : 256 + 128 * kc + 128],
                            rhs=xcmv(cc, n0, 512),
                            start=(cc == 0),
                            stop=(cc == 1),
                        )
                    nc.vector.tensor_copy(qkk[kc][:, n0 : n0 + 512], ps[:])

            def v_proj(i):
                n0 = 128 * i
                ps = psB.tile([128, 512], f32, tag="ps", name="ps")
                for cc in range(2):
                    nc.tensor.matmul(
                        ps[:, :C],
                        lhsT=xcmv(cc, n0, 128),
                        rhs=wv[cc],
                        start=(cc == 0),
                        stop=(cc == 1),
                    )
                va = vt[i][:].rearrange("p (h two v) -> p h two v", two=2, v=32)
                nc.scalar.copy(
                    va[:, :, 0, :],
                    ps[:, :C].rearrange("p (h v) -> p h v", v=32),
                )

            # ---- phase 2: attention ----
            oT = [cpool.tile([128, NQ], bft, tag=f"oT{g}", name=f"oT{g}") for g in range(2)]
            res = [cpool.tile([128, NQ], bft, tag=f"res{t}", name=f"res{t}") for t in range(2)]

            def pair_open():
                # pp rows: [pv_a(0:32)|S_a(32:64)|pv_b(64:96)|S_b(96:128)]
                # zero-matmul opens the bank: clears pending-zero over the
                # full [0:384] so per-chunk 2D-crop PVs can accumulate.
                pp = psPP.tile([128, 512], f32, tag="pp", name="pp")
                nc.tensor.matmul(
                    pp[:, 0:QT],
                    lhsT=zrow[:, 0:128],
                    rhs=zrow[:, 0:QT],
                    start=True,
                    stop=False,
                    skip_group_check=True,
                )
                return pp

            def attn_sem(qt, g, op_i):
                """Scores + exp + mask for one chunk-pair; returns the masked
                attention tile and its geometry for the deferred PV stage."""
                q0row = 8 * qt
                kind = 0 if qt == 0 else 1
                ops = _pairs(qt)
                if True:
                    for grp in [ops[op_i]]:
                        sc = psA.tile([128, 1024], f32, tag="sc", name="sc")
                        offs = []
                        off = 0
                        for ck_id, s in grp:
                            ncs = NCS[s]
                            for hh in range(2):
                                h = 2 * g + hh
                                qc, krow = h // 4, 32 * (h % 4)
                                out_v = sc[
                                    :, 512 * hh + off : 512 * hh + off + 8 * ncs
                                ].rearrange("p (r c) -> p r c", c=ncs)
                                rhs = qkq[qc][krow : krow + 32, :].rearrange(
                                    "p (r c) -> p r c", c=48
                                )[:, q0row : q0row + 8, CLO[s] : CHI[s]]
                                nc.tensor.matmul(
                                    out_v,
                                    lhsT=qkk[qc][
                                        krow : krow + 32,
                                        128 * ck_id : 128 * ck_id + 128,
                                    ],
                                    rhs=rhs,
                                    start=True,
                                    stop=True,
                                    tile_position=(krow, 0),
                                )
                            offs.append(off)
                            off += 8 * ncs
                        ep = off
                        ex = epool.tile([128, 1024], bft, tag="ex", name="ex")
                        sc_v = sc[:].rearrange("p (h q) -> p h q", q=512)[
                            :, :, 0:ep
                        ]
                        ex_v = ex[:, 0 : 2 * ep].rearrange(
                            "p (h q) -> p h q", q=ep
                        )
                        nc.scalar.activation(ex_v, sc_v, AF.Exp, scale=SCALE)
                        ma = apool.tile([128, 1024], bft, tag="ma", name="ma")
                        ma_v = ma[:, 0 : 2 * ep].rearrange(
                            "p (h q) -> p h q", q=ep
                        )
                        mk = msk[
                            :, MOFF[(kind, op_i)] : MOFF[(kind, op_i)] + ep
                        ]
                        # TensorTensor gets the DVE 2x_1p mode (bf16, packed);
                        # a few ops go to the otherwise-idle GpSimd engine.
                        mask_eng = nc.vector
                        mask_eng.tensor_mul(
                            ma_v,
                            ex_v,
                            mk[:, None, :].broadcast_to([128, 2, ep]),
                        )
                        return ma, ep, offs, grp

            def attn_pv(qt, g, op_i, pp, sem):
                ma, ep, offs, grp = sem
                last_op = op_i == len(_pairs(qt)) - 1
                for j, (ck_id, s) in enumerate(grp):
                    ncs = NCS[s]
                    vi = vt[ck_id]
                    for hh in range(2):
                        h = 2 * g + hh
                        out_v = pp[
                            64 * hh : 64 * hh + 64, 0:QT
                        ].rearrange("p (r c) -> p r c", c=48)[
                            :, :, CLO[s] : CHI[s]
                        ]
                        nc.tensor.matmul(
                            out_v,
                            lhsT=vi[:, 64 * h : 64 * h + 64],
                            rhs=ma[
                                :,
                                ep * hh + offs[j] : ep * hh + offs[j] + 8 * ncs,
                            ],
                            start=False,
                            stop=(
                                last_op
                                and j == len(grp) - 1
                                and hh == 1
                            ),
                            skip_group_check=True,
                            tile_position=(0, 64 * hh),
                        )
                if last_op:
                    pair_fin(qt, g, pp)

            def pair_fin(qt, g, pp):
                if True:
                    # normalize: recip sums, shift down 32 partitions onto pv
                    # lanes, multiply, compact to channel order.
                    rc = rpool.tile([128, QT], f32, tag="rc", name="rc")
                    nc.vector.reciprocal(rc[:], pp[:, 0:QT])
                    rcs = rpool.tile([128, QT], f32, tag="rcs", name="rcs")
                    nc.sync.dma_start(rcs[0:96, :], rc[32:128, :])
                    on = rpool.tile([128, QT], bft, tag="on", name="on")
                    nc.vector.tensor_mul(
                        on[0:96, :], pp[0:96, 0:QT], rcs[0:96, :]
                    )
                    nc.sync.dma_start(
                        oT[g // 2][
                            64 * (g % 2) : 64 * (g % 2) + 32,
                            QT * qt : QT * qt + QT,
                        ],
                        on[0:32, :],
                    )
                    nc.sync.dma_start(
                        oT[g // 2][
                            64 * (g % 2) + 32 : 64 * (g % 2) + 64,
                            QT * qt : QT * qt + QT,
                        ],
                        on[64:96, :],
                    )
            def qt_proj_a(qt):
                # out-proj + residual for this qtile's columns
                n0 = QT * qt
                for oc in range(2):
                    ps = psB.tile([128, 512], f32, tag="ps", name="ps")
                    for cc in range(2):
                        nc.tensor.matmul(
                            ps[:, :QT],
                            lhsT=wo[cc][:, 128 * oc : 128 * oc + 128],
                            rhs=oT[cc][:, n0 : n0 + QT],
                            start=(cc == 0),
                            stop=(cc == 1),
                        )
                    nc.vector.tensor_add(
                        res[oc][:, n0 : n0 + QT],
                        ps[:, :QT],
                        xres[oc][:, n0 : n0 + QT],
                    )

            def qt_proj_b(qt):
                # 1x1 conv + bias/relu + store
                n0 = QT * qt
                for oc in range(2):
                    ps = psB.tile([128, 512], f32, tag="ps", name="ps")
                    for cc in range(2):
                        nc.tensor.matmul(
                            ps[:, :QT],
                            lhsT=wc[cc][:, 128 * oc : 128 * oc + 128],
                            rhs=res[cc][:, n0 : n0 + QT],
                            start=(cc == 0),
                            stop=(cc == 1),
                        )
                    ob = opool.tile([128, QT], f32, tag="ob", name="ob")
                    if qt == 2:
                        # tail: Act engine is idle here; doing bias+relu there
                        # takes it off the serial DVE chain
                        nc.scalar.activation(
                            ob[:], ps[:, :QT], AF.Relu,
                            bias=bcr[:, oc : oc + 1],
                        )
                    else:
                        nc.vector.tensor_scalar(
                            ob[:],
                            ps[:, :QT],
                            bcr[:, oc : oc + 1],
                            0.0,
                            OP.add,
                            OP.max,
                        )
                    nc.sync.dma_start(
                        out_d[128 * oc : 128 * oc + 128, n0 : n0 + QT], ob[:]
                    )

            # Software-pipelined emission with a one-stage skew: the NEXT
            # chunk-pair's score matmuls are queued on the PE before the
            # previous pair-op's PV matmuls, so the PE never stalls waiting
            # for the exp+mask of the op it just scored. Phase-1 projection
            # chunks are interleaved so qt0 attention starts as soon as its
            # first chunks are projected.
            segs = []
            segs.append(("fn", lambda: (q_proj(0, tiles=(0,)),
                                        k_proj(0, tiles=(0,)),
                                        v_proj(0), v_proj(1))))
            segs.append(("op", 0, 0, 0))
            segs.append(("fn", lambda: (v_proj(2), v_proj(3))))
            segs.append(("op", 0, 0, 1))
            segs.append(("fn", lambda: (k_proj(0, tiles=(512,)),
                                        v_proj(4), v_proj(5))))
            segs.append(("op", 0, 0, 2))
            for op_i in range(3):
                segs.append(("op", 0, 1, op_i))
            segs.append(("fn", lambda: (q_proj(1, tiles=(0,)),
                                        k_proj(1, tiles=(0, 512)))))
            for g in (2, 3):
                for op_i in range(3):
                    segs.append(("op", 0, g, op_i))
            # deferred phase-1 + qt0 projections woven into qt1's stream
            segs.append(("fn", lambda: (q_proj(0, tiles=(384, 768)),
                                        k_proj(0, tiles=(1024,)))))
            segs.append(("op", 1, 0, 0))
            segs.append(("fn", lambda: (v_proj(6), v_proj(7))))
            segs.append(("op", 1, 0, 1))
            segs.append(("fn", lambda: qt_proj_a(0)))
            segs.append(("op", 1, 0, 2))
            segs.append(("fn", lambda: (v_proj(8), qt_proj_b(0))))
            segs.append(("op", 1, 0, 3))
            segs.append(("fn", lambda: q_proj(1, tiles=(384, 768))))
            segs.append(("op", 1, 0, 4))
            segs.append(("fn", lambda: (k_proj(1, tiles=(1024,)),
                                        v_proj(9))))
            segs.append(("op", 1, 1, 0))
            segs.append(("fn", lambda: (v_proj(10), v_proj(11))))
            for op_i in range(1, 5):
                segs.append(("op", 1, 1, op_i))
            for g in (2, 3):
                for op_i in range(5):
                    segs.append(("op", 1, g, op_i))
            segs.append(("op", 2, 0, 0))
            segs.append(("fn", lambda: qt_proj_a(1)))
            segs.append(("op", 2, 0, 1))
            segs.append(("fn", lambda: qt_proj_b(1)))
            for op_i in range(2, 5):
                segs.append(("op", 2, 0, op_i))
            for g in (1, 2, 3):
                for op_i in range(5):
                    segs.append(("op", 2, g, op_i))

            pps = {}
            for seg in segs:
                if seg[0] == "fn":
                    seg[1]()
                    continue
                _, qt, g, op_i = seg
                if op_i == 0:
                    pps[(qt, g)] = pair_open()
                sem = attn_sem(qt, g, op_i)
                attn_pv(qt, g, op_i, pps[(qt, g)], sem)
            # keep the PE warm across the last pair's normalize chain so the
            # tail projections run at full clock instead of cold p-state.
            pe_warm(26)
            qt_proj_a(2)
            pe_warm(4)
            qt_proj_b(2)

    nc.compile()
    return nc


def _get_program():
    global _PROG
    if _PROG is None:
        _PROG = _build_program()
    return _PROG


def _prep_core_inputs(core, x, in_proj_w, in_proj_b, out_w, out_b, conv_w, conv_b):
    b, half = core // 2, core % 2
    ximg = x[b].reshape(C, H, W)
    if half == 1:
        ximg = ximg[:, ::-1, :]  # row-flip: half-1 becomes half-0 geometry
    bv = in_proj_b[2 * C :].astype(np.float32)
    rbias = out_b.astype(np.float32) + out_w.astype(np.float32) @ bv
    xres = (ximg[:, :ROWS_HALF, :].reshape(C, NQ) + rbias[:, None]).astype(
        bf16
    )
    # chunk-major band: [C, band(4), strip(3), r(8), c(16)]
    xcm = (
        ximg[:, :BAND_ROWS, :]
        .reshape(C, 4, 8, 3, 16)
        .transpose(0, 1, 3, 2, 4)
        .reshape(C, NKCM)
    )
    def halves(a):  # [256, F] -> [128, 2F] with the two halves side by side
        return np.concatenate([a[:128], a[128:]], axis=1)

    wqkT = in_proj_w[: 2 * C].T.astype(np.float32)
    wvT = in_proj_w[2 * C :].T.astype(np.float32)
    xqf = ximg[:, :ROWS_HALF, :].reshape(C, NQ)
    qpk = np.concatenate(
        [
            halves(wqkT),
            xqf[:128, :384], xqf[128:, :384],
            xqf[:128, 384:], xqf[128:, 384:],
        ],
        axis=1,
    ).astype(bf16)
    kv = np.concatenate(
        [
            xcm[:128, :512], xcm[128:, :512],
            halves(wvT),
            xcm[:128, 512:], xcm[128:, 512:],
        ],
        axis=1,
    ).astype(bf16)
    late = np.concatenate(
        [halves(xres.astype(np.float32)), halves(out_w.T), halves(conv_w.T)],
        axis=1,
    ).astype(bf16)
    biases = np.concatenate(
        [in_proj_b[:C].reshape(2, 128).T, conv_b.reshape(2, 128).T], axis=1
    ).astype(np.float32)
    return {
        "qpk": np.ascontiguousarray(qpk),
        "kv": np.ascontiguousarray(kv),
        "late": np.ascontiguousarray(late),
        "biases": np.ascontiguousarray(biases),
        "masks": _masks(),
    }


_MASK_CACHE = {}


def _masks() -> np.ndarray:
    """[128, MTOT] binary pair-masks, shared by every core.

    Column layout matches MOFF: kind 0 (qt0, band deltas 0/+8) then kind 1
    (qt1/qt2, deltas -8/0/+8); each op concatenates its chunk-pair's
    [128, 8*ncs] packed masks. Key partition p = 16*rk + ck.
    """
    if "m" in _MASK_CACHE:
        return _MASK_CACHE["m"]
    cols = []
    for kind, deltas in ((0, (0, 8)), (1, (-8, 0, 8))):
        ch = [(d, s) for d in deltas for s in range(3)]
        for grp in [ch[i : i + 2] for i in range(0, len(ch), 2)]:
            for d, s in grp:
                ncs = NCS[s]
                rk = np.arange(8)
                ck = np.arange(16)
                rq = np.arange(8)
                cq = CLO[s] + np.arange(ncs)
                row_ok = (
                    np.abs((d + rk)[:, None, None, None] - rq[None, None, :, None])
                    <= HALF
                )
                col_ok = (
                    np.abs(
                        (16 * s + ck)[None, :, None, None]
                        - cq[None, None, None, :]
                    )
                    <= HALF
                )
                m = (row_ok & col_ok).reshape(128, 8 * ncs)
                cols.append(m)
    res = np.concatenate(cols, axis=1).astype(bf16)
    assert res.shape == (128, MTOT)
    _MASK_CACHE["m"] = res
    return res


def kernel(**inputs):
    from concourse.bass_utils import run_bass_kernel_spmd

    args = {k: np.asarray(v) for k, v in inputs.items()}
    nc = _get_program()
    in_maps = [
        _prep_core_inputs(core, **args) for core in range(NCORES)
    ]
    res = run_bass_kernel_spmd(nc, in_maps, core_ids=list(range(NCORES)))
    out = np.zeros((B, C, H, W), np.float32)
    for core in range(NCORES):
        b, half = core // 2, core % 2
        o = res.results[core]["out"].reshape(C, ROWS_HALF, W)
        if half == 1:
            o = o[:, ::-1, :]  # undo the row flip
            out[b][:, ROWS_HALF:, :] = o
        else:
            out[b][:, :ROWS_HALF, :] = o
    return out
